# revision 77
# baseline (speedup 1.0000x reference)
"""HSTU layer on 8 trn2 NeuronCores — v3 (fp8 DoubleRow projections).

Sharding: phase 1 tensor-parallel over heads (2 heads/core). The uvqk
projection runs as 3-term error-compensated fp8e4m3 DoubleRow matmuls
(Whi@xhi + Whi@xlo + Wlo@xhi, W pre-scaled by 32 on the host so the lo
residues stay in fp8 normal range; the 1/32 is folded into cos/sin for
q/k, into the silu scale for u, and cancels in the host RMS norm for
v). v is produced token-major directly (lhsT=x, rhs=Wv) so no PE
transpose is needed. RoPE via stream_shuffle; causal silu-attention in
bf16 with valid-width-only silu; AV in flipped orientation. Phase 2
row-parallel output projection, also 3-term fp8 DoubleRow, with the
RMS scale folded into the host-prepared gpre operand. Host does the
RMS reduction + gating between phases.

B=2, S=2048, H=1024, NH=16, HD=64.
"""
import sys
import numpy as np

sys.path.insert(0, "/opt/trn_rl_repo")
import concourse.bass as bass
import concourse.mybir as mybir
import concourse.tile as tile
from concourse.bass_utils import run_bass_kernel_spmd

B, S, H, NH = 2, 2048, 1024, 16
HD = H // NH
EPS = 1e-6
NCORES = 8
R = B * S            # 4096 flattened rows
RC = R // NCORES     # 512 rows per chunk
NCH = R // RC        # 8 chunks (= rounds)
QT = 4               # q-chunks per batch (512 each)
KTB = S // 128       # 16 k-tiles per batch
F32 = mybir.dt.float32
BF16 = mybir.dt.bfloat16
FP8 = mybir.dt.float8e4
DR = mybir.MatmulPerfMode.DoubleRow
AF = mybir.ActivationFunctionType
WSC = 32.0           # host pre-scale on W so fp8 lo residues stay normal

# head-dim permutation making rotate_half intra-quadrant (32) for
# stream_shuffle: quadrant0 = d[0:16]+d[32:48], quadrant1 = d[16:32]+d[48:64]
PERM64 = np.concatenate([np.arange(0, 16), np.arange(32, 48),
                         np.arange(16, 32), np.arange(48, 64)])
SHUF_MASK = list(range(16, 32)) + list(range(16))  # swap halves within quadrant
SIGN64 = np.where(PERM64 < 32, -1.0, 1.0).astype(np.float32)


def legalize_waits(nc, limit=1):
    """neuronxcc here rejects >limit sync waits per instruction; hoist
    excess waits onto preceding NoOps on the same engine."""
    n = 0
    for fn in nc.m.functions:
        for bb in fn.blocks:
            insts = []
            changed = False
            for inst in bb.instructions:
                si = inst.sync_info
                if si is not None and len(si.on_wait) > limit:
                    waits = list(si.on_wait)
                    keep = waits[-limit:]
                    rest = waits[:-limit]
                    for i in range(0, len(rest), limit):
                        insts.append(mybir.InstNoOp(
                            name=f"hoistw-{n}", engine=inst.engine,
                            sync_info=mybir.SyncInfo(on_wait=rest[i:i + limit],
                                                     on_update=[]),
                            bass_nofuse=True))
                        n += 1
                    inst.sync_info = mybir.SyncInfo(on_wait=keep,
                                                    on_update=list(si.on_update))
                    changed = True
                insts.append(inst)
            if changed:
                bb.instructions = insts
    return n


def build_phase1():
    nc = bass.Bass(num_devices=NCORES)
    # x hi/lo: [chunk, p(in-ch within 128-block), ktp, kk, token]
    xhi_ext = nc.dram_tensor("xhi", [NCH, 128, 4, 2, RC], FP8,
                             kind="ExternalInput")
    xlo_ext = nc.dram_tensor("xlo", [NCH, 128, 4, 2, RC], FP8,
                             kind="ExternalInput")
    # W hi/lo: [p, ktp, kk, out-ch(512: u128 v128 q128 k128)]
    whi_ext = nc.dram_tensor("whi", [128, 4, 2, 512], FP8,
                             kind="ExternalInput")
    wlo_ext = nc.dram_tensor("wlo", [128, 4, 2, 512], FP8,
                             kind="ExternalInput")
    cos_ext = nc.dram_tensor("cos2", [128, S], BF16, kind="ExternalInput")
    sin_ext = nc.dram_tensor("sin2", [128, S], BF16, kind="ExternalInput")
    btri_ext = nc.dram_tensor("btri", [128, 128], BF16, kind="ExternalInput")
    # ao[b*4+qc] = [p, (h,j,d)] (q = (b*4+qc)*512 + j*128 + p)
    ao_ext = nc.dram_tensor("ao", [NCH, 128, 512], BF16, kind="ExternalOutput")
    us_ext = nc.dram_tensor("usilu", [128, R], BF16, kind="ExternalOutput")

    with tile.TileContext(nc) as tc:
        with (
            tc.tile_pool(name="const", bufs=1) as constp,
            tc.tile_pool(name="xin", bufs=4) as xin,
            tc.tile_pool(name="big", bufs=1) as big,
            tc.tile_pool(name="rope", bufs=3) as ropep,
            tc.tile_pool(name="attn", bufs=30) as attnp,
            tc.tile_pool(name="aostage", bufs=2) as aostage,
            tc.tile_pool(name="pproj", bufs=2, space="PSUM") as pproj,
            tc.tile_pool(name="pscore", bufs=3, space="PSUM") as pscore,
        ):
            whi_t = constp.tile([128, 4, 2, 512], FP8)
            wlo_t = constp.tile([128, 4, 2, 512], FP8)
            cos_t = constp.tile([128, S], BF16)
            sin_t = constp.tile([128, S], BF16)
            btri_t = constp.tile([128, 128], BF16)
            # first ktp slice of whi alone so the first matmul starts early;
            # wlo goes on sync right after chunk 0 so the lo-terms of the
            # first groups aren't starved.
            nc.sync.dma_start(whi_t[:, 0:1], whi_ext[:, 0:1])
            nc.scalar.dma_start(whi_t[:, 1:4], whi_ext[:, 1:4])

            qrot = big.tile([128, R], BF16, tag="qrot", name="qrot")
            krot = big.tile([128, R], BF16, tag="krot", name="krot")
            us_t = big.tile([128, R], BF16, tag="us", name="us")
            vn = [big.tile([128, KTB, 128], BF16, tag=f"vn{b}", name=f"vn{b}")
                  for b in range(B)]

            TERMS = lambda xh, xl: ((whi_t, xh), (whi_t, xl), (wlo_t, xh))

            projst = {}

            def qk_parts(r):
                # x DMA + q/k projection + RoPE for chunk r as thunks:
                # [dma, g2 x3 terms + rope, g3 x3 terms + rope]
                b, qc = r // QT, r % QT
                st = projst.setdefault(r, {})
                s0 = qc * RC

                def dma():
                    st["xh"] = xh = xin.tile([128, 4, 2, RC], FP8, tag="xh", name="xh")
                    st["xl"] = xl = xin.tile([128, 4, 2, RC], FP8, tag="xl", name="xl")
                    if r == 0:
                        # fine-grained, need-ordered pieces so the term
                        # sequence (hi*hi, hi*lo, lo*hi) never starves and
                        # cos/sin land in time for the first RoPE
                        nc.sync.dma_start(xh[:, 0:1], xhi_ext[0][:, 0:1])
                        nc.sync.dma_start(xh[:, 1:4], xhi_ext[0][:, 1:4])
                        nc.scalar.dma_start(xl[:, 0:1], xlo_ext[0][:, 0:1])
                        nc.scalar.dma_start(xl[:, 1:4], xlo_ext[0][:, 1:4])
                        nc.sync.dma_start(wlo_t[:, 0:2], wlo_ext[:, 0:2])
                        nc.scalar.dma_start(cos_t[:, 0:RC], cos_ext[:, 0:RC])
                        nc.sync.dma_start(wlo_t[:, 2:4], wlo_ext[:, 2:4])
                        nc.scalar.dma_start(sin_t[:, 0:RC], sin_ext[:, 0:RC])
                        nc.scalar.dma_start(cos_t[:, RC:2 * RC],
                                            cos_ext[:, RC:2 * RC])
                        nc.scalar.dma_start(sin_t[:, RC:2 * RC],
                                            sin_ext[:, RC:2 * RC])
                        nc.scalar.dma_start(btri_t[:], btri_ext[:])
                        nc.scalar.dma_start(cos_t[:, 2 * RC:],
                                            cos_ext[:, 2 * RC:])
                        nc.scalar.dma_start(sin_t[:, 2 * RC:],
                                            sin_ext[:, 2 * RC:])
                    else:
                        nc.sync.dma_start(xh[:], xhi_ext[r])
                        nc.sync.dma_start(xl[:], xlo_ext[r])
                parts = [dma]

                def gterm(g, ti):
                    def f():
                        if ti == 0:
                            st[g] = pproj.tile([128, RC], F32, name="psg")
                        wt, xt = TERMS(st["xh"], st["xl"])[ti]
                        for ktp in range(4):
                            nc.tensor.matmul(
                                st[g][:],
                                wt[:, ktp, :, g * 128:(g + 1) * 128],
                                xt[:, ktp, :, :],
                                start=(ti == 0 and ktp == 0),
                                stop=(ti == 2 and ktp == 3),
                                perf_mode=DR)
                    return f

                def rope(g):
                    def f():
                        ps = st[g]
                        dst = qrot if g == 2 else krot
                        sh = ropep.tile([128, RC], F32, tag="sh", name="sh")
                        t1 = ropep.tile([128, RC], BF16, tag="t1", name="t1")
                        t2 = ropep.tile([128, RC], BF16, tag="t2", name="t2")
                        dsl = slice(r * RC, r * RC + RC)
                        csl = slice(s0, s0 + RC)
                        # DVE handles the PSUM reads; the idle Pool engine
                        # does the SBUF-only half so two ropes pipeline
                        nc.vector.stream_shuffle(sh[:], ps[:], SHUF_MASK)
                        nc.vector.tensor_mul(t1[:], ps[:], cos_t[:, csl])
                        nc.gpsimd.tensor_mul(t2[:], sh[:], sin_t[:, csl])
                        nc.gpsimd.tensor_add(dst[:, dsl], t1[:], t2[:])
                    return f

                def gall(g):
                    t0, t1, t2 = gterm(g, 0), gterm(g, 1), gterm(g, 2)
                    rp = rope(g)

                    def f():
                        t0(); t1(); t2(); rp()
                    return f
                parts += [gall(2), gall(3)]
                return parts

            def vu_parts(r):
                # v (token-major) + u projection for chunk r as thunks:
                # [v x4 tb, u x3 terms + silu]; deferred one round after
                # qk_parts so late Act-bound rounds still have PE filler.
                b, qc = r // QT, r % QT
                st = projst[r]
                parts = []


                def gterm(g, ti):
                    def f():
                        if ti == 0:
                            st[g] = pproj.tile([128, RC], F32, name="psg")
                        wt, xt = TERMS(st["xh"], st["xl"])[ti]
                        for ktp in range(4):
                            nc.tensor.matmul(
                                st[g][:],
                                wt[:, ktp, :, g * 128:(g + 1) * 128],
                                xt[:, ktp, :, :],
                                start=(ti == 0 and ktp == 0),
                                stop=(ti == 2 and ktp == 3),
                                perf_mode=DR)
                    return f

                def vtb(tb):
                    def f():
                        if tb == 0:
                            st["v"] = pproj.tile([128, RC], F32, name="psg")
                        ps = st["v"]
                        for wt, xt in TERMS(st["xh"], st["xl"]):
                            for ktp in range(4):
                                nc.tensor.matmul(
                                    ps[:, 128 * tb:128 * tb + 128],
                                    xt[:, ktp, :, 128 * tb:128 * tb + 128],
                                    wt[:, ktp, :, 128:256],
                                    start=(wt is whi_t and xt is st["xh"]
                                           and ktp == 0),
                                    stop=(wt is wlo_t and ktp == 3),
                                    perf_mode=DR)
                        if tb == 3:
                            nc.vector.tensor_copy(
                                vn[b][:, 4 * qc:4 * qc + 4, :], ps[:])
                    return f
                parts += [vtb(0), vtb(1), vtb(2), vtb(3)]

                def ustage():
                    # raw u (x32) to SBUF on DVE; the host applies silu —
                    # keeps the contended Activation engine on score silus
                    nc.vector.tensor_copy(us_t[:, r * RC:(r + 1) * RC],
                                          st[0][:])
                def uall():
                    gterm(0, 0)(); gterm(0, 1)(); gterm(0, 2)(); ustage()
                parts += [uall]

                def done():
                    projst.pop(r)
                parts.append(done)
                return parts

            def scores_parts(r, ats):
                # scores + silu per (h, ktp) tile; fills ats[h] lists
                b, qc = r // QT, r % QT
                qf = r * RC

                def tilef(h, ktp):
                    def f():
                        sps = pscore.tile([128, 1024], F32, name="sps")
                        rels = []
                        sks = []
                        for i in range(2):
                            kt = 2 * ktp + i
                            kf = b * S + kt * 128
                            rel = kt - 4 * qc
                            # columns q < 128*rel are fully masked: skip
                            sk = 128 * max(rel, 0)
                            sks.append(sk)
                            nc.tensor.matmul(
                                sps[:, 512 * i + sk:512 * i + 512],
                                krot[64 * h:64 * h + 64, kf:kf + 128],
                                qrot[64 * h:64 * h + 64, qf + sk:qf + RC],
                                start=True, stop=True)
                            if rel >= 0:
                                rels.append((i, rel))
                        at = attnp.tile([128, 1024], BF16, name="at")
                        # silu only the valid columns; 1/sqrt(HD) folded
                        # into the activation scale. A single call covering
                        # a small masked hole beats two calls (185ns each).
                        if sks[0] == 0 and sks[1] <= 128:
                            nc.scalar.activation(at[:], sps[:], AF.Silu,
                                                 scale=0.125)
                        else:
                            for i in range(2):
                                if sks[i] < 512:
                                    rg = slice(512 * i + sks[i], 512 * (i + 1))
                                    nc.scalar.activation(at[:, rg], sps[:, rg],
                                                         AF.Silu, scale=0.125)
                        # diagonal 128x128 blocks: multiplicative causal
                        # mask. The last unit's masks go on the (by then
                        # idle) DVE so the final AV chain is short.
                        meng = nc.vector if r == 4 else nc.gpsimd
                        for i, rel in rels:
                            off = 512 * i + 128 * rel
                            meng.tensor_mul(at[:, off:off + 128],
                                            at[:, off:off + 128],
                                            btri_t[:])
                        ats[h].append(at)
                    return f
                parts, weights = [], []
                for h in range(2):
                    for ktp in range(2 * qc + 2):
                        parts.append(tilef(h, ktp))
                        sks = [128 * max(2 * ktp + i - 4 * qc, 0)
                               for i in range(2)]
                        if sks[0] == 0 and sks[1] <= 128:
                            w, calls = 1024, 1
                        else:
                            w = sum(512 - sk for sk in sks if sk < 512)
                            calls = sum(1 for sk in sks if sk < 512)
                        weights.append(w * 0.833 + calls * 185)

                def lhs_of(h, kt, j):
                    at = ats[h][kt // 2]
                    c = 512 * (kt % 2) + 128 * j
                    return at[:, c:c + 128]
                return parts, weights, lhs_of

            def av_parts(r, lhs_of, split=False):
                # AV for unit r as per-(h,j) accumulation-block thunks
                # sharing one PSUM bank, then a copy+DMA thunk.
                b, qc = r // QT, r % QT
                st = {}

                def block(h, j):
                    def f():
                        if h == 0 and j == 0:
                            st["pv"] = pproj.tile([128, 512], F32,
                                                  name="psg")
                            st["ao"] = aostage.tile([128, 512], BF16, name="aos")
                        pv = st["pv"]
                        qt = 4 * qc + j
                        off = 256 * h + 64 * j
                        for kt in range(qt + 1):
                            nc.tensor.matmul(pv[:, off:off + 64],
                                             lhs_of(h, kt, j),
                                             vn[b][:, kt, 64 * h:64 * h + 64],
                                             start=(kt == 0), stop=(kt == qt))
                        if split and j == 3:
                            hs = slice(256 * h, 256 * h + 256)
                            nc.vector.tensor_copy(st["ao"][:, hs],
                                                  pv[:, hs])
                            nc.sync.dma_start(ao_ext[r][:, hs],
                                              st["ao"][:, hs])
                    return f
                parts = [block(h, j) for h in range(2) for j in range(4)]
                if not split:
                    def drain():
                        nc.vector.tensor_copy(st["ao"][:], st["pv"][:])
                        nc.sync.dma_start(ao_ext[r], st["ao"][:])
                    parts.append(drain)
                return parts

            # Global schedule. Score tiles (the Activation-feeding stream)
            # are emitted greedily; filler (proj k-steps, RoPE, AV blocks)
            # is emitted whenever the Act backlog exceeds a threshold, so
            # the PE always has ready work queued while the Activation
            # engine drains silus — and the spare filler lands in the
            # Act-heavy late units instead of the idle early rounds.
            # Units end with the small (b1,qc0) unit for a short tail.
            units = [0, 1, 2, 3, 5, 6, 7, 4]
            for p in qk_parts(0) + qk_parts(1):
                p()

            def usdma(k):
                def f():
                    nc.sync.dma_start(
                        us_ext[:, 2 * k * RC:(2 * k + 2) * RC],
                        us_t[:, 2 * k * RC:(2 * k + 2) * RC])
                return f

            # S stream: (thunk, act_ns, pe_ns, unit_idx) in unit order
            sstream = []
            unit_lhs = []
            unit_last_tile = {}
            ntiles = []
            for ui, u in enumerate(units):
                ats = [[], []]
                parts, weights, lhs_of = scores_parts(u, ats)
                unit_lhs.append(lhs_of)
                ntiles.append(len(parts))
                for p, w in zip(parts, weights):
                    sstream.append((p, w, 430, ui))
                unit_last_tile[ui] = len(sstream) - 1

            # F stream: (thunk, pe_ns, barrier_s_index) in a legal total
            # order: qk/vu in chunk order, av in unit order, us after vu.
            fstream = []

            def addf(parts, pe_each, barrier=-1):
                for p in parts:
                    fstream.append((p, pe_each, barrier))

            av_done_idx = {}
            fseq = [("qk", 2), ("vu", 0), ("qk", 3), ("vu", 1), ("us", 0),
                    ("av", 0), ("qk", 4), ("vu", 2), ("av", 1),
                    ("qk", 5), ("vu", 3), ("us", 1), ("av", 2),
                    ("qk", 6), ("vu", 4), ("av", 3),
                    ("qk", 7), ("vu", 5), ("us", 2), ("av", 4),
                    ("vu", 6), ("av", 5), ("vu", 7), ("us", 3),
                    ("av", 6), ("av", 7)]
            for kind, a in fseq:
                if kind == "qk":
                    addf(qk_parts(a), 500)
                elif kind == "vu":
                    addf(vu_parts(a), 400)
                elif kind == "us":
                    addf([usdma(a)], 0)
                else:       # av of units[a], after that unit's last tile
                    ui = a
                    u = units[ui]
                    parts = av_parts(u, unit_lhs[ui], split=(ui == 7))
                    avpe = (sum(range(4 * (u % QT) + 1,
                                      4 * (u % QT) + 5)) * 2 * 27) // 8
                    for p in parts:
                        fstream.append((p, avpe, unit_last_tile[ui]))
                    av_done_idx[len(fstream) - 1] = ui

            TH = 1200.0
            backlog = 0.0
            alive = 0
            av_released = 0
            si = fi = 0
            while si < len(sstream) or fi < len(fstream):
                f_ok = fi < len(fstream) and (fstream[fi][2] < si or si >= len(sstream))
                s_ok = si < len(sstream) and alive < 28
                early = si <= unit_last_tile[1]
                if s_ok and (backlog <= TH or early or not f_ok):
                    p, act_ns, pe_ns, ui = sstream[si]
                    p()
                    backlog += act_ns - pe_ns
                    alive += 1
                    si += 1
                elif f_ok:
                    p, pe_ns, _ = fstream[fi]
                    p()
                    backlog = max(0.0, backlog - pe_ns)
                    if fi in av_done_idx:
                        alive -= ntiles[av_done_idx[fi]]
                    fi += 1
                else:
                    # blocked both ways shouldn't happen; emit S to advance
                    p, act_ns, pe_ns, ui = sstream[si]
                    p()
                    backlog += act_ns - pe_ns
                    alive += 1
                    si += 1
    legalize_waits(nc, limit=1)
    return nc


def build_phase2():
    # row-parallel out-proj: out = gpre @ WoT in 3-term fp8 DoubleRow.
    # gpre has silu(u) gating and the RMS row-scale folded in on the host;
    # WoT carries gate_w and the x32 fp8 scale (undone on the host).
    nc = bass.Bass(num_devices=NCORES)
    # g laid out token-block-major so each 128-token block is one
    # contiguous DMA piece: [t, p(ch), ktp, kk, tok128]
    ghi_ext = nc.dram_tensor("ghi", [4, 128, 4, 2, 128], FP8,
                             kind="ExternalInput")
    glo_ext = nc.dram_tensor("glo", [4, 128, 4, 2, 128], FP8,
                             kind="ExternalInput")
    whi_ext = nc.dram_tensor("wohi", [128, 4, 2, H], FP8, kind="ExternalInput")
    wlo_ext = nc.dram_tensor("wolo", [128, 4, 2, H], FP8, kind="ExternalInput")
    out_ext = nc.dram_tensor("out", [4, 128, H], BF16, kind="ExternalOutput")

    with tile.TileContext(nc) as tc:
        with (
            tc.tile_pool(name="sb", bufs=1) as sb,
            tc.tile_pool(name="ostage", bufs=4) as ostage,
            tc.tile_pool(name="pmm", bufs=1, space="PSUM") as pmm,
        ):
            ghi_t = sb.tile([128, 4, 4, 2, 128], FP8, tag="ghi", name="ghi")
            glo_t = sb.tile([128, 4, 4, 2, 128], FP8, tag="glo", name="glo")
            whi_t = sb.tile([128, 4, 2, H], FP8, tag="wohi", name="wohi")
            wlo_t = sb.tile([128, 4, 2, H], FP8, tag="wolo", name="wolo")
            # need-ordered, few big pieces (per-queue DMA issue costs
            # ~1.25us each, so piece count is as important as order)
            nc.sync.dma_start(ghi_t[:, 0], ghi_ext[0])
            nc.scalar.dma_start(whi_t[:, 0:1], whi_ext[:, 0:1])
            nc.sync.dma_start(ghi_t[:, 1:4],
                              ghi_ext[1:4].rearrange("t p k two o -> p t k two o"))
            nc.scalar.dma_start(whi_t[:, 1:2], whi_ext[:, 1:2])
            nc.sync.dma_start(glo_t[:, 0], glo_ext[0])
            nc.scalar.dma_start(whi_t[:, 2:4], whi_ext[:, 2:4])
            nc.sync.dma_start(glo_t[:, 1:4],
                              glo_ext[1:4].rearrange("t p k two o -> p t k two o"))
            nc.scalar.dma_start(wlo_t[:, 0:2], wlo_ext[:, 0:2])
            nc.scalar.dma_start(wlo_t[:, 2:4], wlo_ext[:, 2:4])

            pss = [pmm.tile([128, 512], F32, tag=f"ps{t}{oh}",
                            name=f"ps{t}{oh}")
                   for t in range(4) for oh in range(2)]

            def mm(gt, wt, ktp, t, oh, start=False, stop=False):
                nc.tensor.matmul(
                    pss[2 * t + oh][:],
                    gt[:, t, ktp, :, :],
                    wt[:, ktp, :, 512 * oh:512 * oh + 512],
                    start=start, stop=stop, perf_mode=DR)

            # hi*hi then lo*hi for all blocks (hi/glo DMAs), then per-block
            # hi*lo stages: block t finishes and drains while block t+1
            # computes, so the out-DMAs overlap the remaining matmuls.
            for ktp in range(4):
                for oh in range(2):
                    for t in range(4):
                        mm(ghi_t, whi_t, ktp, t, oh, start=(ktp == 0))
            for ktp in range(4):
                for t in range(4):
                    for oh in range(2):
                        mm(glo_t, whi_t, ktp, t, oh)
            for t in range(4):
                o_t = ostage.tile([128, 1024], BF16, name="o_t")
                for ktp in range(4):
                    for oh in range(2):
                        mm(ghi_t, wlo_t, ktp, t, oh, stop=(ktp == 3))
                nc.vector.tensor_copy(o_t[:, 0:512], pss[2 * t][:])
                nc.scalar.activation(o_t[:, 512:1024], pss[2 * t + 1][:],
                                     AF.Copy)
                eng = nc.sync if t % 2 == 0 else nc.scalar
                eng.dma_start(out_ext[t], o_t[:])
    legalize_waits(nc, limit=1)
    return nc


_NC1 = None
_NC2 = None


def _q8split(a, f8):
    hi = a.astype(f8)
    lo = (a - hi.astype(np.float32)).astype(f8)
    return hi, lo


def kernel(x, cos, sin, attn_mask, W_uvqk, b_uvqk, gate_w, W_out, b_out):
    global _NC1, _NC2
    import ml_dtypes
    bf = ml_dtypes.bfloat16
    f8 = ml_dtypes.float8_e4m3
    xf = np.asarray(x, np.float32).reshape(R, H)
    # ---- host prep, phase 1 ----
    # x -> [chunk, p, ktp, kk, tok] fp8 hi/lo
    xr = xf.reshape(NCH, RC, 4, 2, 128)
    xhi, xlo = _q8split(xr, f8)
    xhi = np.ascontiguousarray(xhi.transpose(0, 4, 2, 3, 1))
    xlo = np.ascontiguousarray(xlo.transpose(0, 4, 2, 3, 1))

    perm2 = np.concatenate([PERM64, PERM64 + 64])          # per head pair
    cosT = np.asarray(cos, np.float32)[0].T                # [HD, S]
    sinT = np.asarray(sin, np.float32)[0].T
    cosP = cosT[PERM64] / WSC
    sinP = sinT[PERM64] * SIGN64[:, None] / WSC
    cos2 = np.ascontiguousarray(np.tile(cosP, (2, 1))).astype(bf)   # [128, S]
    sin2 = np.ascontiguousarray(np.tile(sinP, (2, 1))).astype(bf)

    ki = np.arange(128)[:, None]
    qj = np.arange(128)[None, :]
    btri = (qj >= ki).astype(np.float32).astype(bf)   # multiplicative mask

    Wg = np.asarray(W_uvqk, np.float32)
    bq = np.asarray(b_uvqk, np.float32)
    assert np.abs(bq).max() == 0.0, "nonzero b_uvqk not folded"
    maps1 = []
    for c in range(NCORES):
        dsl = np.arange(128 * c, 128 * c + 128)
        rows_u = dsl
        rows_v = H + dsl
        rows_q = 2 * H + 128 * c + perm2
        rows_k = 3 * H + 128 * c + perm2
        Wc = Wg[np.concatenate([rows_u, rows_v, rows_q, rows_k])]  # [512, H]
        Wcs = np.ascontiguousarray(Wc.T) * WSC                     # [H, 512]
        Wr = Wcs.reshape(4, 2, 128, 512)
        whi, wlo = _q8split(Wr, f8)
        whi = np.ascontiguousarray(whi.transpose(2, 0, 1, 3))
        wlo = np.ascontiguousarray(wlo.transpose(2, 0, 1, 3))
        maps1.append({"xhi": xhi, "xlo": xlo, "whi": whi, "wlo": wlo,
                      "cos2": cos2, "sin2": sin2, "btri": btri})

    if _NC1 is None:
        _NC1 = build_phase1()
    r1 = run_bass_kernel_spmd(_NC1, maps1, list(range(NCORES)))

    # ---- host mid: gating product, RMS scale folded into gpre ----
    # ao result [8, 128, 2, 4, 64] -> [R, 128]: row = rd*512 + j*128 + p
    aos, uss = [], []
    for c in range(NCORES):
        a = np.asarray(r1.results[c]["ao"]).astype(np.float32) / WSC
        a = a.reshape(NCH, 128, 2, 4, 64)
        aos.append(np.ascontiguousarray(
            a.transpose(0, 3, 1, 2, 4)).reshape(R, 128))
        ur = np.asarray(r1.results[c]["usilu"]).astype(np.float32) / WSC
        uss.append(ur / (1.0 + np.exp(-ur)))
    ao = np.stack(aos)                                     # [8, R, 128]
    sumsq = np.einsum("crd,crd->r", ao, ao)
    invr = 1.0 / np.sqrt(sumsq / H + EPS)                  # [R]
    gpre = np.concatenate(
        [ao[c] * uss[c].T for c in range(NCORES)], axis=1)  # [R, H]
    gpre *= invr[:, None]
    ghi, glo = _q8split(gpre, f8)

    WoS = np.ascontiguousarray((np.asarray(W_out, np.float32)
                                * np.asarray(gate_w, np.float32)[None, :]).T
                               ) * WSC                      # [H(in), H(out)]
    Wor = WoS.reshape(4, 2, 128, H)
    wohi, wolo = _q8split(Wor, f8)
    wohi = np.ascontiguousarray(wohi.transpose(2, 0, 1, 3))
    wolo = np.ascontiguousarray(wolo.transpose(2, 0, 1, 3))
    maps2 = []
    for c in range(NCORES):
        rows = slice(RC * c, RC * c + RC)
        g8h = np.ascontiguousarray(
            ghi[rows].reshape(4, 128, 4, 2, 128).transpose(0, 4, 2, 3, 1))
        g8l = np.ascontiguousarray(
            glo[rows].reshape(4, 128, 4, 2, 128).transpose(0, 4, 2, 3, 1))
        maps2.append({"ghi": g8h, "glo": g8l, "wohi": wohi, "wolo": wolo})

    if _NC2 is None:
        _NC2 = build_phase2()
    r2 = run_bass_kernel_spmd(_NC2, maps2, list(range(NCORES)))

    mm = np.concatenate([np.asarray(r2.results[c]["out"]).astype(np.float32)
                         .reshape(RC, H) for c in range(NCORES)], axis=0)
    out = xf + np.asarray(b_out, np.float32)[None, :] + mm / WSC
    return out.reshape(B, S, H).astype(x.dtype)
